# revision 15
# baseline (speedup 1.0000x reference)
"""
Trainium2 Bass kernel for the Decoder_RNN_Simple problem (v2).

Math (per flat-batch element b, reference semantics):
  hidden0 = tanh(W_z0 @ z0 + b_z0)                       # [256]
  cur0 = 0
  for t in 0..199:
    x = [cur, tps[t]]                                    # [65]
    gx = W_ih @ x + b_ih ; gh = W_hh @ hidden + b_hh     # [768]
    r = sig(gx_r + gh_r); z = sig(gx_z + gh_z)
    n = tanh(gx_n + r * gh_n)
    h' = (1-z)*n + z*h ; pred = W_out @ h' + b_out       # [64]

Mapping (data-parallel over the flat batch of 8192 across 8 cores,
1024 rows per core, 2 independent 512-column streams per core;
[gates, batch] on-chip layout, bf16 everywhere except PSUM/biases):

  - cur_t = pred_{t-1} = W_out @ h_t + b_out for t>=1 is folded into
    ALL gates: r/z use W_eff = W_hh_rz + W_ih_rz@W_out; the n-gate x
    part uses W_xnh = W_ih_n@W_out so pred never feeds back on-chip.
    The n gate needs xn separate from hn: n = tanh(xn + r*(hn+b_hhn)).
  - t-dependent bias terms are host-precomputed [gate, 200] tables and
    applied through the ACT bias operand.
  - The n-gate sum (xn + t1) is formed in PSUM: the DVE STT writes
    t1 = (hn+b_hhn)*r into a PSUM bank whose has_written bits are
    pre-primed, then the xn matmuls accumulate onto it (start=False).
  - pred = W_out@h' + (b_out added on host) is DMA'd straight from
    PSUM to DRAM; it is not on the recurrence critical path.
"""

import sys

_TRN = "/opt/trn_rl_repo"
if _TRN not in sys.path:
    sys.path.insert(0, _TRN)

import numpy as np

import concourse.bass as bass
import concourse.mybir as mybir
import concourse.tile as tile
from concourse.vector_clock import ScopedClock
from concourse.bass_utils import run_bass_kernel_spmd

N_CORES = 8
LATENT = 128
OUT_DIM = 64
N_GRU = 256
N_TP = 200
B_FULL = 64 * 128
B_LOC = B_FULL // N_CORES  # 1024
SW = 512  # batch columns per stream
NS = 2  # streams
F32 = mybir.dt.float32
BF16 = mybir.dt.bfloat16
AF = mybir.ActivationFunctionType
ALU = mybir.AluOpType

# If the PE-accumulate-onto-DVE-written-PSUM trick fails on HW, set True
# to fall back to an explicit DVE add for t2 = t1 + xn.
USE_TT_ADD = False
# Engine for the blend's d = h - n: "gpsimd" or "dve". gpsimd shares its
# SBUF port with the DVE and inflates concurrent DVE ops ~2x — keep "dve".
SUB_ENGINE = "dve"


# walrus rejects sem waits carried on the kernel-tail Drain instruction
# ("Too many sync wait commands"); move them onto NOPs, one wait each.
def _patched_drain_and_barrier(self, tick_clock, wait_clock):
    carrier = self.nc.sync.nop()
    wait_clock.add_sem_waits(carrier.ins, ScopedClock({None: tick_clock.global_clock}))
    si = carrier.ins.sync_info
    waits = list(si.on_wait) if si is not None else []
    if len(waits) > 1:
        si.on_wait = waits[:1]
        rest = waits[1:]
        while rest:
            extra = self.nc.sync.nop()
            extra.ins.sync_info = mybir.SyncInfo(on_wait=rest[:1], on_update=[])
            rest = rest[1:]
    self.nc.sync.drain()
    self.nc.all_engine_barrier()
    popped = self.nc._tile_sem_poison_stack.pop()
    assert popped is self._sem_poison
    self.nc.clear_and_free_semaphores(list(self.sems.allocated().values()))
    self.nc.all_engine_barrier()


tile.TileContext._drain_and_barrier = _patched_drain_and_barrier


def _split_waits(nc, maxw=1):
    """This walrus rejects instructions carrying more than a couple of sem
    waits; move the excess onto same-engine NOPs inserted just before."""
    k = 0
    for f in nc.m.functions:
        for bb in f.blocks:
            insts = bb.instructions
            out = []
            changed = False
            for inst in insts:
                si = inst.sync_info
                waits = list(si.on_wait) if si is not None else []
                if len(waits) > maxw:
                    si.on_wait = waits[-maxw:]
                    excess = waits[:-maxw]
                    while excess:
                        chunk, excess = excess[:maxw], excess[maxw:]
                        nop = mybir.InstNoOp(name=f"waitsplit_{k}", ins=[], outs=[])
                        k += 1
                        nop.engine = inst.engine
                        nop.sync_info = mybir.SyncInfo(on_wait=chunk, on_update=[])
                        out.append(nop)
                    changed = True
                out.append(inst)
            if changed:
                bb.instructions = out
    return k


def _build_module():
    nc = bass.Bass("TRN2", target_bir_lowering=False, debug=False, num_devices=N_CORES)

    def inp(name, shape, dt=F32):
        return nc.dram_tensor(name, shape, dt, kind="ExternalInput").ap()

    d = {
        # bf16 operands
        "z0t": inp("z0t", [LATENT, B_LOC], BF16),
        "wz0t": inp("wz0t", [LATENT, N_GRU], BF16),
        # [256, 768]: cols 0:256 r(eff) 256:512 z(eff) 512:768 hn
        "w1t": inp("w1t", [N_GRU, 3 * N_GRU], BF16),
        # [64, 256]: xn weights vs pred (pred-feedback form)
        "wxnt": inp("wxnt", [OUT_DIM, N_GRU], BF16),
        # [256, 512]: raw W_hh r,z for step 0
        "w0t": inp("w0t", [N_GRU, 2 * N_GRU], BF16),
        "woutt": inp("woutt", [N_GRU, OUT_DIM], BF16),
        # f32 bias tables
        "brz": inp("brz", [2 * N_GRU, N_TP]),
        "bxn": inp("bxn", [N_GRU, N_TP]),
        "bhhn": inp("bhhn", [N_GRU, 1]),
        "bz0": inp("bz0", [N_GRU, 1]),
    }
    out = nc.dram_tensor("out", [N_TP, OUT_DIM, B_LOC], BF16, kind="ExternalOutput").ap()

    with tile.TileContext(nc) as tc:
        _emit(nc, tc, d, out)
    n = _split_waits(nc, maxw=1)
    print(f"[kernel] split {n} excess sem-waits onto NOPs", flush=True)
    return nc


def _emit(nc, tc, d, out):
    with (
        tc.tile_pool(name="const", bufs=1) as cp,
        tc.tile_pool(name="work", bufs=2) as wp,
        tc.tile_pool(name="prz", bufs=3, space="PSUM") as przp,
        tc.tile_pool(name="phn", bufs=2, space="PSUM") as phnp,
        tc.tile_pool(name="pnx", bufs=2, space="PSUM") as pnxp,
        tc.tile_pool(name="ppr", bufs=1, space="PSUM") as pprp,
    ):
        def const_tile(name, shape, dt=F32):
            t = cp.tile(shape, dt, tag=name, name=name + "_c")
            nc.sync.dma_start(t[:], d[name][:])
            return t

        def const_rows(name, shape, r0, tag, dt=F32):
            t = cp.tile(shape, dt, tag=tag, name=tag + "_c")
            nc.sync.dma_start(t[:], d[name][r0 : r0 + shape[0], :])
            return t

        wz0 = const_tile("wz0t", [LATENT, N_GRU], BF16)
        wxn = const_tile("wxnt", [OUT_DIM, N_GRU], BF16)
        w1 = [const_rows("w1t", [128, 3 * N_GRU], 128 * k, f"w1_{k}", BF16) for k in range(2)]
        w0 = [const_rows("w0t", [128, 2 * N_GRU], 128 * k, f"w0_{k}", BF16) for k in range(2)]
        wout = [const_rows("woutt", [128, OUT_DIM], 128 * k, f"wout_{k}", BF16) for k in range(2)]
        brz = [const_rows("brz", [128, N_TP], 128 * g, f"brz_{g}") for g in range(4)]
        bxn = [const_rows("bxn", [128, N_TP], 128 * c, f"bxn_{c}") for c in range(2)]
        bhhn = [const_rows("bhhn", [128, 1], 128 * c, f"bhhn_{c}") for c in range(2)]
        bz0 = [const_rows("bz0", [128, 1], 128 * c, f"bz0_{c}") for c in range(2)]

        z0sb = wp.tile([LATENT, B_LOC], BF16, tag="z0", bufs=1)
        nc.sync.dma_start(z0sb[:], d["z0t"][:])

        # ---- initial hidden: h[s][c] = tanh(Wz0 @ z0T + b_z0)  [128, SW]
        h = [[None, None] for _ in range(NS)]
        for s in range(NS):
            bs = slice(s * SW, (s + 1) * SW)
            for c in range(2):
                p = przp.tile([128, SW], F32, tag="prz", name="p0")
                nc.tensor.matmul(p[:], wz0[:, c * 128 : (c + 1) * 128], z0sb[:, bs],
                                 start=True, stop=True)
                hc = wp.tile([128, SW], BF16, tag=f"h{c}_{s}", name="h0")
                nc.scalar.activation(hc[:], p[:], AF.Tanh, bias=bz0[c][:, 0:1])
                h[s][c] = hc

        # Prime the nx psum banks' has_written bits with dummy matmuls so
        # later start=False matmuls accumulate onto DVE-written data.
        primed = []
        if not USE_TT_ADD:
            for i in range(2 * 2 * NS):  # cover all rotating bufs of the tag
                pp = pnxp.tile([128, SW], F32, tag="pnx", name="prime")
                nc.tensor.matmul(pp[:], w1[0][:, 0:128], z0sb[:, 0:SW],
                                 start=True, stop=True)
                primed.append(pp)

        sub_eng = nc.gpsimd if SUB_ENGINE == "gpsimd" else nc.vector

        def emit_pred(t, s, hs):
            """pred(t) = W_out @ h'(t); b_out is added host-side."""
            bs = slice(s * SW, (s + 1) * SW)
            p = pprp.tile([OUT_DIM, SW], F32, tag="ppr", name="ppr")
            nc.tensor.matmul(p[:], wout[0][:, :], hs[0][:],
                             start=True, stop=False)
            nc.tensor.matmul(p[:], wout[1][:, :], hs[1][:],
                             start=False, stop=True)
            pr = wp.tile([OUT_DIM, SW], BF16, tag=f"pred_{s}", name="pr")
            if s == 0:
                nc.vector.tensor_copy(pr[:], p[:])
            else:
                nc.scalar.copy(pr[:], p[:])
            nc.sync.dma_start(out[t][:, bs], pr[:])
            return pr

        # Software-pipelined main loop (modulo-scheduled): per step the PE
        # queue is [pred+rz+hn](s0), [pred+rz](s1), xn(s0), [hn](s1),
        # xn(s1); the pointwise tail of each stream overlaps the other
        # stream's matmul block. pred(t-1) is issued at the top of step
        # t's block (same readiness point as the step-t gate matmuls).
        def emit_rz(t, s, wk):
            rzt = [None] * 4
            for g in range(4):  # r0 r1 z0 z1
                col = slice(g * 128, (g + 1) * 128)
                p = przp.tile([128, SW], F32, tag="prz", name="prz")
                nc.tensor.matmul(p[:], wk[0][:, col], h[s][0][:],
                                 start=True, stop=False)
                nc.tensor.matmul(p[:], wk[1][:, col], h[s][1][:],
                                 start=False, stop=True)
                gg = wp.tile([128, SW], BF16, tag=f"g{g}_{s}", name="gact")
                nc.scalar.activation(gg[:], p[:], AF.Sigmoid,
                                     bias=brz[g][:, t : t + 1])
                rzt[g] = gg
            return rzt[:2], rzt[2:]

        def emit_hn_stt(t, s, rt, first):
            """hn matmuls + STT into the nx psum (or SBUF t1 at t=0)."""
            srcs = [None, None]
            for c in range(2):
                col = slice(512 + c * 128, 512 + (c + 1) * 128)
                p = phnp.tile([128, SW], F32, tag="phn", name="phn")
                nc.tensor.matmul(p[:], w1[0][:, col], h[s][0][:],
                                 start=True, stop=False)
                nc.tensor.matmul(p[:], w1[1][:, col], h[s][1][:],
                                 start=False, stop=True)
                if first:
                    dst = wp.tile([128, SW], BF16, tag=f"t1_{c}_{s}", name="t1")
                elif USE_TT_ADD:
                    dst = wp.tile([128, SW], BF16, tag=f"t1_{c}_{s}", name="t1")
                else:
                    dst = pnxp.tile([128, SW], F32, tag="pnx", name="pnx")
                nc.vector.scalar_tensor_tensor(
                    dst[:], p[:], bhhn[c][:, 0:1], rt[c][:], ALU.add, ALU.mult)
                srcs[c] = dst
            return srcs

        def emit_xn(srcs, pr):
            """xn = W_xn @ pred(t-1), accumulated onto the STT psum."""
            outs = [None, None]
            for c in range(2):
                xcol = slice(c * 128, (c + 1) * 128)
                if USE_TT_ADD:
                    px = pnxp.tile([128, SW], F32, tag="pnx", name="pnx")
                    nc.tensor.matmul(px[:], wxn[:, xcol], pr[:],
                                     start=True, stop=True)
                    t2 = wp.tile([128, SW], F32, tag=f"t2_{c}", name="t2")
                    nc.vector.tensor_tensor(t2[:], srcs[c][:], px[:], ALU.add)
                    outs[c] = t2
                else:
                    nc.tensor.matmul(srcs[c][:], wxn[:, xcol], pr[:],
                                     start=False, stop=True,
                                     skip_group_check=True)
                    outs[c] = srcs[c]
            return outs

        def emit_tail(t, s, srcs, zt, h_new):
            """tanh + blend h' = n + z*(h-n)."""
            for c in range(2):
                nt = wp.tile([128, SW], BF16, tag=f"n_{c}_{s}", name="nt")
                nc.scalar.activation(nt[:], srcs[c][:], AF.Tanh,
                                     bias=bxn[c][:, t : t + 1])
                dt_ = wp.tile([128, SW], BF16, tag=f"d_{c}_{s}", name="dt")
                sub_eng.tensor_tensor(dt_[:], h[s][c][:], nt[:], ALU.subtract)
                e = wp.tile([128, SW], BF16, tag=f"e_{c}_{s}", name="et")
                nc.vector.tensor_tensor(e[:], zt[c][:], dt_[:], ALU.mult)
                hc = wp.tile([128, SW], BF16, tag=f"h{c}_{s}", name="hn2")
                nc.vector.tensor_tensor(hc[:], e[:], nt[:], ALU.add)
                h_new[s][c] = hc

        for t in range(N_TP):
            first = t == 0
            wk = w0 if first else w1
            h_new = [[None, None] for _ in range(NS)]

            # stream 0 matmul block
            pr0 = None if first else emit_pred(t - 1, 0, h[0])
            rt0, zt0 = emit_rz(t, 0, wk)
            srcs0 = emit_hn_stt(t, 0, rt0, first)

            # stream 1 matmul block, with stream 0's xn wedged mid-block
            pr1 = None if first else emit_pred(t - 1, 1, h[1])
            rt1, zt1 = emit_rz(t, 1, wk)
            if not first:
                srcs0 = emit_xn(srcs0, pr0)
            srcs1 = emit_hn_stt(t, 1, rt1, first)
            if not first:
                srcs1 = emit_xn(srcs1, pr1)

            # pointwise tails (overlap the next step's matmul blocks)
            emit_tail(t, 0, srcs0, zt0, h_new)
            emit_tail(t, 1, srcs1, zt1, h_new)
            h = h_new

        for s in range(NS):
            emit_pred(N_TP - 1, s, h[s])


_CACHE = {}


def _prep_host(z0, tps_to_pred, W_z0, b_z0, W_ih, b_ih, W_hh, b_hh, W_out, b_out):
    import ml_dtypes

    f = np.float32
    bf = ml_dtypes.bfloat16
    z0 = np.asarray(z0, f)
    tps = np.asarray(tps_to_pred, f)
    W_z0, b_z0 = np.asarray(W_z0, f), np.asarray(b_z0, f)
    W_ih, b_ih = np.asarray(W_ih, f), np.asarray(b_ih, f)
    W_hh, b_hh = np.asarray(W_hh, f), np.asarray(b_hh, f)
    W_out, b_out = np.asarray(W_out, f), np.asarray(b_out, f)

    G2 = 2 * N_GRU
    Wihp = W_ih[:, :OUT_DIM]  # [768, 64]
    wt = W_ih[:, OUT_DIM]  # [768]
    Weff_rz = W_hh[:G2] + Wihp[:G2] @ W_out  # [512, 256]
    W1 = np.concatenate([Weff_rz, W_hh[G2:]], axis=0)  # [768, 256]
    w1t = np.ascontiguousarray(W1.T).astype(bf)  # [256, 768]
    wxnt = np.ascontiguousarray(Wihp[G2:].T).astype(bf)  # [64, 256]
    w0t = np.ascontiguousarray(W_hh[:G2].T).astype(bf)  # [256, 512]
    woutt = np.ascontiguousarray(W_out.T).astype(bf)  # [256, 64]
    wz0t = np.ascontiguousarray(W_z0.T).astype(bf)  # [128, 256]

    cb = Wihp @ b_out  # [768]
    bias_all = b_ih[:, None] + wt[:, None] * tps[None, :]  # [768, 200]
    brz = bias_all[:G2] + b_hh[:G2, None]
    brz[:, 1:] += cb[:G2, None]
    bxn = bias_all[G2:].copy()
    bxn[:, 1:] += cb[G2:, None]

    shared = {
        "wz0t": wz0t,
        "w1t": w1t,
        "wxnt": wxnt,
        "w0t": w0t,
        "woutt": woutt,
        "brz": np.ascontiguousarray(brz, f),
        "bxn": np.ascontiguousarray(bxn, f),
        "bhhn": np.ascontiguousarray(b_hh[G2:].reshape(N_GRU, 1)),
        "bz0": np.ascontiguousarray(b_z0.reshape(N_GRU, 1)),
    }
    z0f = z0.reshape(B_FULL, LATENT)
    in_maps = []
    for i in range(N_CORES):
        m = dict(shared)
        m["z0t"] = np.ascontiguousarray(z0f[i * B_LOC : (i + 1) * B_LOC].T).astype(bf)
        in_maps.append(m)
    return in_maps, b_out


def _run(in_maps, **spmd_kwargs):
    if "nc" not in _CACHE:
        _CACHE["nc"] = _build_module()
    return run_bass_kernel_spmd(_CACHE["nc"], in_maps, list(range(N_CORES)), **spmd_kwargs)


def _gather(res, b_out):
    outp = np.empty((B_FULL, N_TP, OUT_DIM), np.float32)
    for i in range(N_CORES):
        o = np.asarray(res.results[i]["out"]).astype(np.float32)  # [200, 64, 1024]
        outp[i * B_LOC : (i + 1) * B_LOC] = o.transpose(2, 0, 1)
    outp += b_out[None, None, :]
    return outp.reshape(64, 128, N_TP, OUT_DIM)


def kernel(**inputs):
    in_maps, b_out = _prep_host(**inputs)
    res = _run(in_maps)
    return _gather(res, b_out)


def kernel_profiled(**inputs):
    """Like kernel(), but requests an NTFF trace; returns (output, results)."""
    in_maps, b_out = _prep_host(**inputs)
    res = _run(in_maps, trace=True)
    return _gather(res, b_out), res


# revision 18
# speedup vs baseline: 1.3136x; 1.3136x over previous
"""
Trainium2 Bass kernel for the Decoder_RNN_Simple problem (v2).

Math (per flat-batch element b, reference semantics):
  hidden0 = tanh(W_z0 @ z0 + b_z0)                       # [256]
  cur0 = 0
  for t in 0..199:
    x = [cur, tps[t]]                                    # [65]
    gx = W_ih @ x + b_ih ; gh = W_hh @ hidden + b_hh     # [768]
    r = sig(gx_r + gh_r); z = sig(gx_z + gh_z)
    n = tanh(gx_n + r * gh_n)
    h' = (1-z)*n + z*h ; pred = W_out @ h' + b_out       # [64]

Mapping (data-parallel over the flat batch of 8192 across 8 cores,
1024 rows per core, 2 independent 512-column streams per core;
[gates, batch] on-chip layout, bf16 everywhere except PSUM/biases):

  - cur_t = pred_{t-1} = W_out @ h_t + b_out for t>=1 is folded into
    ALL gates: r/z use W_eff = W_hh_rz + W_ih_rz@W_out; the n-gate x
    part uses W_xnh = W_ih_n@W_out so pred never feeds back on-chip.
    The n gate needs xn separate from hn: n = tanh(xn + r*(hn+b_hhn)).
  - t-dependent bias terms are host-precomputed [gate, 200] tables and
    applied through the ACT bias operand.
  - The n-gate sum (xn + t1) is formed in PSUM: the DVE STT writes
    t1 = (hn+b_hhn)*r into a PSUM bank whose has_written bits are
    pre-primed, then the xn matmuls accumulate onto it (start=False).
  - pred = W_out@h' + (b_out added on host) is DMA'd straight from
    PSUM to DRAM; it is not on the recurrence critical path.
"""

import sys

_TRN = "/opt/trn_rl_repo"
if _TRN not in sys.path:
    sys.path.insert(0, _TRN)

import numpy as np

import concourse.bass as bass
import concourse.mybir as mybir
import concourse.tile as tile
from concourse.vector_clock import ScopedClock
from concourse.bass_utils import run_bass_kernel_spmd

N_CORES = 8
LATENT = 128
OUT_DIM = 64
N_GRU = 256
N_TP = 200
B_FULL = 64 * 128
B_LOC = B_FULL // N_CORES  # 1024
SW = 512  # batch columns per stream
NS = 2  # streams
F32 = mybir.dt.float32
BF16 = mybir.dt.bfloat16
AF = mybir.ActivationFunctionType
ALU = mybir.AluOpType

# If the PE-accumulate-onto-DVE-written-PSUM trick fails on HW, set True
# to fall back to an explicit DVE add for t2 = t1 + xn.
USE_TT_ADD = False
# Engine for the blend's d = h - n: "gpsimd" or "dve". gpsimd shares its
# SBUF port with the DVE and inflates concurrent DVE ops ~2x — keep "dve".
SUB_ENGINE = "dve"


# walrus rejects sem waits carried on the kernel-tail Drain instruction
# ("Too many sync wait commands"); move them onto NOPs, one wait each.
def _patched_drain_and_barrier(self, tick_clock, wait_clock):
    carrier = self.nc.sync.nop()
    wait_clock.add_sem_waits(carrier.ins, ScopedClock({None: tick_clock.global_clock}))
    si = carrier.ins.sync_info
    waits = list(si.on_wait) if si is not None else []
    if len(waits) > 1:
        si.on_wait = waits[:1]
        rest = waits[1:]
        while rest:
            extra = self.nc.sync.nop()
            extra.ins.sync_info = mybir.SyncInfo(on_wait=rest[:1], on_update=[])
            rest = rest[1:]
    self.nc.sync.drain()
    self.nc.all_engine_barrier()
    popped = self.nc._tile_sem_poison_stack.pop()
    assert popped is self._sem_poison
    self.nc.clear_and_free_semaphores(list(self.sems.allocated().values()))
    self.nc.all_engine_barrier()


tile.TileContext._drain_and_barrier = _patched_drain_and_barrier


def _split_waits(nc, maxw=1):
    """This walrus rejects instructions carrying more than a couple of sem
    waits; move the excess onto same-engine NOPs inserted just before."""
    k = 0
    for f in nc.m.functions:
        for bb in f.blocks:
            insts = bb.instructions
            out = []
            changed = False
            for inst in insts:
                si = inst.sync_info
                waits = list(si.on_wait) if si is not None else []
                if len(waits) > maxw:
                    si.on_wait = waits[-maxw:]
                    excess = waits[:-maxw]
                    while excess:
                        chunk, excess = excess[:maxw], excess[maxw:]
                        nop = mybir.InstNoOp(name=f"waitsplit_{k}", ins=[], outs=[])
                        k += 1
                        nop.engine = inst.engine
                        nop.sync_info = mybir.SyncInfo(on_wait=chunk, on_update=[])
                        out.append(nop)
                    changed = True
                out.append(inst)
            if changed:
                bb.instructions = out
    return k


def _build_module():
    nc = bass.Bass("TRN2", target_bir_lowering=False, debug=False, num_devices=N_CORES)

    def inp(name, shape, dt=F32):
        return nc.dram_tensor(name, shape, dt, kind="ExternalInput").ap()

    d = {
        # bf16 operands
        "z0t": inp("z0t", [LATENT, B_LOC], BF16),
        "wz0t": inp("wz0t", [LATENT, N_GRU], BF16),
        # [256, 768]: cols 0:256 r(eff) 256:512 z(eff) 512:768 hn
        "w1t": inp("w1t", [N_GRU, 3 * N_GRU], BF16),
        # [64, 256]: xn weights vs pred (pred-feedback form)
        "wxnt": inp("wxnt", [OUT_DIM, N_GRU], BF16),
        # [256, 512]: raw W_hh r,z for step 0
        "w0t": inp("w0t", [N_GRU, 2 * N_GRU], BF16),
        "woutt": inp("woutt", [N_GRU, OUT_DIM], BF16),
        # f32 bias tables
        "brz": inp("brz", [2 * N_GRU, N_TP]),
        "bxn": inp("bxn", [N_GRU, N_TP]),
        "bhhn": inp("bhhn", [N_GRU, 1]),
        "bz0": inp("bz0", [N_GRU, 1]),
    }
    out = nc.dram_tensor("out", [N_TP, OUT_DIM, B_LOC], BF16, kind="ExternalOutput").ap()

    with tile.TileContext(nc) as tc:
        _emit(nc, tc, d, out)
    n = _split_waits(nc, maxw=1)
    print(f"[kernel] split {n} excess sem-waits onto NOPs", flush=True)
    return nc


def _emit(nc, tc, d, out):
    with (
        tc.tile_pool(name="const", bufs=1) as cp,
        tc.tile_pool(name="work", bufs=2) as wp,
        tc.tile_pool(name="prz", bufs=3, space="PSUM") as przp,
        tc.tile_pool(name="pnx", bufs=4, space="PSUM") as pnxp,
        tc.tile_pool(name="ppr", bufs=1, space="PSUM") as pprp,
    ):
        def const_tile(name, shape, dt=F32):
            t = cp.tile(shape, dt, tag=name, name=name + "_c")
            nc.sync.dma_start(t[:], d[name][:])
            return t

        def const_rows(name, shape, r0, tag, dt=F32):
            t = cp.tile(shape, dt, tag=tag, name=tag + "_c")
            nc.sync.dma_start(t[:], d[name][r0 : r0 + shape[0], :])
            return t

        wz0 = const_tile("wz0t", [LATENT, N_GRU], BF16)
        wxn = const_tile("wxnt", [OUT_DIM, N_GRU], BF16)
        w1 = [const_rows("w1t", [128, 3 * N_GRU], 128 * k, f"w1_{k}", BF16) for k in range(2)]
        w0 = [const_rows("w0t", [128, 2 * N_GRU], 128 * k, f"w0_{k}", BF16) for k in range(2)]
        wout = [const_rows("woutt", [128, OUT_DIM], 128 * k, f"wout_{k}", BF16) for k in range(2)]
        brz = [const_rows("brz", [128, N_TP], 128 * g, f"brz_{g}") for g in range(4)]
        bxn = [const_rows("bxn", [128, N_TP], 128 * c, f"bxn_{c}") for c in range(2)]
        bhhn = [const_rows("bhhn", [128, 1], 128 * c, f"bhhn_{c}") for c in range(2)]
        bz0 = [const_rows("bz0", [128, 1], 128 * c, f"bz0_{c}") for c in range(2)]

        z0sb = wp.tile([LATENT, B_LOC], BF16, tag="z0", bufs=1)
        nc.sync.dma_start(z0sb[:], d["z0t"][:])

        # ---- initial hidden: h[s][c] = tanh(Wz0 @ z0T + b_z0)  [128, SW]
        h = [[None, None] for _ in range(NS)]
        for s in range(NS):
            bs = slice(s * SW, (s + 1) * SW)
            for c in range(2):
                p = przp.tile([128, SW], F32, tag="prz", name="p0")
                nc.tensor.matmul(p[:], wz0[:, c * 128 : (c + 1) * 128], z0sb[:, bs],
                                 start=True, stop=True)
                hc = wp.tile([128, SW], BF16, tag=f"h{c}_{s}", name="h0")
                nc.scalar.activation(hc[:], p[:], AF.Tanh, bias=bz0[c][:, 0:1])
                h[s][c] = hc

        sub_eng = nc.gpsimd if SUB_ENGINE == "gpsimd" else nc.vector

        def emit_pred(t, s, hs):
            """pred(t) = W_out @ h'(t); b_out is added host-side."""
            bs = slice(s * SW, (s + 1) * SW)
            p = pprp.tile([OUT_DIM, SW], F32, tag="ppr", name="ppr")
            nc.tensor.matmul(p[:], wout[0][:, :], hs[0][:],
                             start=True, stop=False)
            nc.tensor.matmul(p[:], wout[1][:, :], hs[1][:],
                             start=False, stop=True)
            pr = wp.tile([OUT_DIM, SW], BF16, tag=f"pred_{s}", name="pr")
            if s == 0:
                nc.vector.tensor_copy(pr[:], p[:])
            else:
                nc.scalar.copy(pr[:], p[:])
            nc.sync.dma_start(out[t][:, bs], pr[:])
            return pr

        # Software-pipelined main loop (modulo-scheduled): per step the PE
        # queue is [pred+rz+hn](s0), [pred+rz](s1), xn(s0), [hn](s1),
        # xn(s1); the pointwise tail of each stream overlaps the other
        # stream's matmul block. pred(t-1) is issued at the top of step
        # t's block (same readiness point as the step-t gate matmuls).
        def emit_rz(t, s, wk):
            rzt = [None] * 4
            for g in range(4):  # r0 r1 z0 z1
                col = slice(g * 128, (g + 1) * 128)
                p = przp.tile([128, SW], F32, tag="prz", name="prz")
                nc.tensor.matmul(p[:], wk[0][:, col], h[s][0][:],
                                 start=True, stop=False)
                nc.tensor.matmul(p[:], wk[1][:, col], h[s][1][:],
                                 start=False, stop=True)
                gg = wp.tile([128, SW], BF16, tag=f"g{g}_{s}", name="gact")
                nc.scalar.activation(gg[:], p[:], AF.Sigmoid,
                                     bias=brz[g][:, t : t + 1])
                rzt[g] = gg
            return rzt[:2], rzt[2:]

        def emit_hn_stt(t, s, rt, first):
            """hn matmuls straight into the nx psum; STT runs IN-PLACE on
            that bank: px = (px + b_hhn) * r. The hn matmuls set the
            has_written bits so the later xn matmul accumulates."""
            srcs = [None, None]
            for c in range(2):
                col = slice(512 + c * 128, 512 + (c + 1) * 128)
                px = pnxp.tile([128, SW], F32, tag="pnx", name="pnx")
                nc.tensor.matmul(px[:], w1[0][:, col], h[s][0][:],
                                 start=True, stop=False)
                nc.tensor.matmul(px[:], w1[1][:, col], h[s][1][:],
                                 start=False, stop=(True if first else False),
                                 skip_group_check=True)
                nc.vector.scalar_tensor_tensor(
                    px[:], px[:], bhhn[c][:, 0:1], rt[c][:], ALU.add, ALU.mult)
                srcs[c] = px
            return srcs

        def emit_xn(srcs, pr):
            """xn = W_xn @ pred(t-1), accumulated onto the in-place STT psum."""
            outs = [None, None]
            for c in range(2):
                xcol = slice(c * 128, (c + 1) * 128)
                nc.tensor.matmul(srcs[c][:], wxn[:, xcol], pr[:],
                                 start=False, stop=True,
                                 skip_group_check=True)
                outs[c] = srcs[c]
            return outs

        def emit_tail(t, s, srcs, zt, h_new):
            """tanh + blend h' = n + z*(h-n)."""
            for c in range(2):
                nt = wp.tile([128, SW], BF16, tag=f"n_{c}_{s}", name="nt")
                nc.scalar.activation(nt[:], srcs[c][:], AF.Tanh,
                                     bias=bxn[c][:, t : t + 1])
                dt_ = wp.tile([128, SW], BF16, tag=f"d_{c}_{s}", name="dt")
                sub_eng.tensor_tensor(dt_[:], h[s][c][:], nt[:], ALU.subtract)
                e = wp.tile([128, SW], BF16, tag=f"e_{c}_{s}", name="et")
                nc.vector.tensor_tensor(e[:], zt[c][:], dt_[:], ALU.mult)
                hc = wp.tile([128, SW], BF16, tag=f"h{c}_{s}", name="hn2")
                nc.vector.tensor_tensor(hc[:], e[:], nt[:], ALU.add)
                h_new[s][c] = hc

        for t in range(N_TP):
            first = t == 0
            wk = w0 if first else w1
            h_new = [[None, None] for _ in range(NS)]

            # stream 0 matmul block
            pr0 = None if first else emit_pred(t - 1, 0, h[0])
            rt0, zt0 = emit_rz(t, 0, wk)
            srcs0 = emit_hn_stt(t, 0, rt0, first)

            # stream 1 matmul block, with stream 0's xn wedged mid-block
            pr1 = None if first else emit_pred(t - 1, 1, h[1])
            rt1, zt1 = emit_rz(t, 1, wk)
            if not first:
                srcs0 = emit_xn(srcs0, pr0)
            srcs1 = emit_hn_stt(t, 1, rt1, first)
            if not first:
                srcs1 = emit_xn(srcs1, pr1)

            # pointwise tails (overlap the next step's matmul blocks)
            emit_tail(t, 0, srcs0, zt0, h_new)
            emit_tail(t, 1, srcs1, zt1, h_new)
            h = h_new

        for s in range(NS):
            emit_pred(N_TP - 1, s, h[s])


_CACHE = {}


def _prep_host(z0, tps_to_pred, W_z0, b_z0, W_ih, b_ih, W_hh, b_hh, W_out, b_out):
    import ml_dtypes

    f = np.float32
    bf = ml_dtypes.bfloat16
    z0 = np.asarray(z0, f)
    tps = np.asarray(tps_to_pred, f)
    W_z0, b_z0 = np.asarray(W_z0, f), np.asarray(b_z0, f)
    W_ih, b_ih = np.asarray(W_ih, f), np.asarray(b_ih, f)
    W_hh, b_hh = np.asarray(W_hh, f), np.asarray(b_hh, f)
    W_out, b_out = np.asarray(W_out, f), np.asarray(b_out, f)

    G2 = 2 * N_GRU
    Wihp = W_ih[:, :OUT_DIM]  # [768, 64]
    wt = W_ih[:, OUT_DIM]  # [768]
    Weff_rz = W_hh[:G2] + Wihp[:G2] @ W_out  # [512, 256]
    W1 = np.concatenate([Weff_rz, W_hh[G2:]], axis=0)  # [768, 256]
    w1t = np.ascontiguousarray(W1.T).astype(bf)  # [256, 768]
    wxnt = np.ascontiguousarray(Wihp[G2:].T).astype(bf)  # [64, 256]
    w0t = np.ascontiguousarray(W_hh[:G2].T).astype(bf)  # [256, 512]
    woutt = np.ascontiguousarray(W_out.T).astype(bf)  # [256, 64]
    wz0t = np.ascontiguousarray(W_z0.T).astype(bf)  # [128, 256]

    cb = Wihp @ b_out  # [768]
    bias_all = b_ih[:, None] + wt[:, None] * tps[None, :]  # [768, 200]
    brz = bias_all[:G2] + b_hh[:G2, None]
    brz[:, 1:] += cb[:G2, None]
    bxn = bias_all[G2:].copy()
    bxn[:, 1:] += cb[G2:, None]

    shared = {
        "wz0t": wz0t,
        "w1t": w1t,
        "wxnt": wxnt,
        "w0t": w0t,
        "woutt": woutt,
        "brz": np.ascontiguousarray(brz, f),
        "bxn": np.ascontiguousarray(bxn, f),
        "bhhn": np.ascontiguousarray(b_hh[G2:].reshape(N_GRU, 1)),
        "bz0": np.ascontiguousarray(b_z0.reshape(N_GRU, 1)),
    }
    z0f = z0.reshape(B_FULL, LATENT)
    in_maps = []
    for i in range(N_CORES):
        m = dict(shared)
        m["z0t"] = np.ascontiguousarray(z0f[i * B_LOC : (i + 1) * B_LOC].T).astype(bf)
        in_maps.append(m)
    return in_maps, b_out


def _run(in_maps, **spmd_kwargs):
    if "nc" not in _CACHE:
        _CACHE["nc"] = _build_module()
    return run_bass_kernel_spmd(_CACHE["nc"], in_maps, list(range(N_CORES)), **spmd_kwargs)


def _gather(res, b_out):
    outp = np.empty((B_FULL, N_TP, OUT_DIM), np.float32)
    for i in range(N_CORES):
        o = np.asarray(res.results[i]["out"]).astype(np.float32)  # [200, 64, 1024]
        outp[i * B_LOC : (i + 1) * B_LOC] = o.transpose(2, 0, 1)
    outp += b_out[None, None, :]
    return outp.reshape(64, 128, N_TP, OUT_DIM)


def kernel(**inputs):
    in_maps, b_out = _prep_host(**inputs)
    res = _run(in_maps)
    return _gather(res, b_out)


def kernel_profiled(**inputs):
    """Like kernel(), but requests an NTFF trace; returns (output, results)."""
    in_maps, b_out = _prep_host(**inputs)
    res = _run(in_maps, trace=True)
    return _gather(res, b_out), res


# revision 22
# speedup vs baseline: 1.3230x; 1.0071x over previous
"""
Trainium2 Bass kernel for the Decoder_RNN_Simple problem (v2).

Math (per flat-batch element b, reference semantics):
  hidden0 = tanh(W_z0 @ z0 + b_z0)                       # [256]
  cur0 = 0
  for t in 0..199:
    x = [cur, tps[t]]                                    # [65]
    gx = W_ih @ x + b_ih ; gh = W_hh @ hidden + b_hh     # [768]
    r = sig(gx_r + gh_r); z = sig(gx_z + gh_z)
    n = tanh(gx_n + r * gh_n)
    h' = (1-z)*n + z*h ; pred = W_out @ h' + b_out       # [64]

Mapping (data-parallel over the flat batch of 8192 across 8 cores,
1024 rows per core, 2 independent 512-column streams per core;
[gates, batch] on-chip layout, bf16 everywhere except PSUM/biases):

  - cur_t = pred_{t-1} = W_out @ h_t + b_out for t>=1 is folded into
    ALL gates: r/z use W_eff = W_hh_rz + W_ih_rz@W_out; the n-gate x
    part uses W_xnh = W_ih_n@W_out so pred never feeds back on-chip.
    The n gate needs xn separate from hn: n = tanh(xn + r*(hn+b_hhn)).
  - t-dependent bias terms are host-precomputed [gate, 200] tables and
    applied through the ACT bias operand.
  - The n-gate sum (xn + t1) is formed in PSUM: the DVE STT writes
    t1 = (hn+b_hhn)*r into a PSUM bank whose has_written bits are
    pre-primed, then the xn matmuls accumulate onto it (start=False).
  - pred = W_out@h' + (b_out added on host) is DMA'd straight from
    PSUM to DRAM; it is not on the recurrence critical path.
"""

import sys

_TRN = "/opt/trn_rl_repo"
if _TRN not in sys.path:
    sys.path.insert(0, _TRN)

import numpy as np

import concourse.bass as bass
import concourse.mybir as mybir
import concourse.tile as tile
from concourse.vector_clock import ScopedClock
from concourse.bass_utils import run_bass_kernel_spmd

N_CORES = 8
LATENT = 128
OUT_DIM = 64
N_GRU = 256
N_TP = 200
B_FULL = 64 * 128
B_LOC = B_FULL // N_CORES  # 1024
SW = 512  # batch columns per stream
NS = 2  # streams
F32 = mybir.dt.float32
BF16 = mybir.dt.bfloat16
AF = mybir.ActivationFunctionType
ALU = mybir.AluOpType

# If the PE-accumulate-onto-DVE-written-PSUM trick fails on HW, set True
# to fall back to an explicit DVE add for t2 = t1 + xn.
USE_TT_ADD = False
# Engine for the blend's d = h - n: "gpsimd" or "dve". gpsimd shares its
# SBUF port with the DVE and inflates concurrent DVE ops ~2x — keep "dve".
SUB_ENGINE = "dve"


# walrus rejects sem waits carried on the kernel-tail Drain instruction
# ("Too many sync wait commands"); move them onto NOPs, one wait each.
def _patched_drain_and_barrier(self, tick_clock, wait_clock):
    carrier = self.nc.sync.nop()
    wait_clock.add_sem_waits(carrier.ins, ScopedClock({None: tick_clock.global_clock}))
    si = carrier.ins.sync_info
    waits = list(si.on_wait) if si is not None else []
    if len(waits) > 1:
        si.on_wait = waits[:1]
        rest = waits[1:]
        while rest:
            extra = self.nc.sync.nop()
            extra.ins.sync_info = mybir.SyncInfo(on_wait=rest[:1], on_update=[])
            rest = rest[1:]
    self.nc.sync.drain()
    self.nc.all_engine_barrier()
    popped = self.nc._tile_sem_poison_stack.pop()
    assert popped is self._sem_poison
    self.nc.clear_and_free_semaphores(list(self.sems.allocated().values()))
    self.nc.all_engine_barrier()


tile.TileContext._drain_and_barrier = _patched_drain_and_barrier


def _split_waits(nc, maxw=1):
    """This walrus rejects instructions carrying more than a couple of sem
    waits; move the excess onto same-engine NOPs inserted just before."""
    k = 0
    for f in nc.m.functions:
        for bb in f.blocks:
            insts = bb.instructions
            out = []
            changed = False
            for inst in insts:
                si = inst.sync_info
                waits = list(si.on_wait) if si is not None else []
                if len(waits) > maxw:
                    si.on_wait = waits[-maxw:]
                    excess = waits[:-maxw]
                    while excess:
                        chunk, excess = excess[:maxw], excess[maxw:]
                        nop = mybir.InstNoOp(name=f"waitsplit_{k}", ins=[], outs=[])
                        k += 1
                        nop.engine = inst.engine
                        nop.sync_info = mybir.SyncInfo(on_wait=chunk, on_update=[])
                        out.append(nop)
                    changed = True
                out.append(inst)
            if changed:
                bb.instructions = out
    return k


def _build_module():
    nc = bass.Bass("TRN2", target_bir_lowering=False, debug=False, num_devices=N_CORES)

    def inp(name, shape, dt=F32):
        return nc.dram_tensor(name, shape, dt, kind="ExternalInput").ap()

    d = {
        # bf16 operands
        "z0t": inp("z0t", [LATENT, B_LOC], BF16),
        "wz0t": inp("wz0t", [LATENT, N_GRU], BF16),
        # [256, 768]: cols 0:256 r(eff) 256:512 z(eff) 512:768 hn
        "w1t": inp("w1t", [N_GRU, 3 * N_GRU], BF16),
        # [64, 256]: xn weights vs pred (pred-feedback form)
        "wxnt": inp("wxnt", [OUT_DIM, N_GRU], BF16),
        # [256, 512]: raw W_hh r,z for step 0
        "w0t": inp("w0t", [N_GRU, 2 * N_GRU], BF16),
        "woutt": inp("woutt", [N_GRU, OUT_DIM], BF16),
        # f32 bias tables
        "brz": inp("brz", [2 * N_GRU, N_TP]),
        "bxn": inp("bxn", [N_GRU, N_TP]),
        "bhhn": inp("bhhn", [N_GRU, 1]),
        "bz0": inp("bz0", [N_GRU, 1]),
    }
    out = nc.dram_tensor("out", [N_TP, OUT_DIM, B_LOC], BF16, kind="ExternalOutput").ap()

    with tile.TileContext(nc) as tc:
        _emit(nc, tc, d, out)
    n = _split_waits(nc, maxw=1)
    print(f"[kernel] split {n} excess sem-waits onto NOPs", flush=True)
    return nc


def _emit(nc, tc, d, out):
    with (
        tc.tile_pool(name="const", bufs=1) as cp,
        tc.tile_pool(name="work", bufs=2) as wp,
        tc.tile_pool(name="prz", bufs=3, space="PSUM") as przp,
        tc.tile_pool(name="pnx", bufs=4, space="PSUM") as pnxp,
        tc.tile_pool(name="ppr", bufs=1, space="PSUM") as pprp,
    ):
        def const_tile(name, shape, dt=F32):
            t = cp.tile(shape, dt, tag=name, name=name + "_c")
            nc.sync.dma_start(t[:], d[name][:])
            return t

        def const_rows(name, shape, r0, tag, dt=F32):
            t = cp.tile(shape, dt, tag=tag, name=tag + "_c")
            nc.sync.dma_start(t[:], d[name][r0 : r0 + shape[0], :])
            return t

        wz0 = const_tile("wz0t", [LATENT, N_GRU], BF16)
        wxn = const_tile("wxnt", [OUT_DIM, N_GRU], BF16)
        w1 = [const_rows("w1t", [128, 3 * N_GRU], 128 * k, f"w1_{k}", BF16) for k in range(2)]
        w0 = [const_rows("w0t", [128, 2 * N_GRU], 128 * k, f"w0_{k}", BF16) for k in range(2)]
        wout = [const_rows("woutt", [128, OUT_DIM], 128 * k, f"wout_{k}", BF16) for k in range(2)]
        brz = [const_rows("brz", [128, N_TP], 128 * g, f"brz_{g}") for g in range(4)]
        bxn = [const_rows("bxn", [128, N_TP], 128 * c, f"bxn_{c}") for c in range(2)]
        bhhn = [const_rows("bhhn", [128, 1], 128 * c, f"bhhn_{c}") for c in range(2)]
        bz0 = [const_rows("bz0", [128, 1], 128 * c, f"bz0_{c}") for c in range(2)]

        z0sb = wp.tile([LATENT, B_LOC], BF16, tag="z0", bufs=1)
        nc.sync.dma_start(z0sb[:], d["z0t"][:])

        # ---- initial hidden: h[s][c] = tanh(Wz0 @ z0T + b_z0)  [128, SW]
        h = [[None, None] for _ in range(NS)]
        for s in range(NS):
            bs = slice(s * SW, (s + 1) * SW)
            for c in range(2):
                p = przp.tile([128, SW], F32, tag="prz", name="p0")
                nc.tensor.matmul(p[:], wz0[:, c * 128 : (c + 1) * 128], z0sb[:, bs],
                                 start=True, stop=True)
                hc = wp.tile([128, SW], BF16, tag=f"h{c}_{s}", name="h0")
                nc.scalar.activation(hc[:], p[:], AF.Tanh, bias=bz0[c][:, 0:1])
                h[s][c] = hc

        sub_eng = nc.gpsimd if SUB_ENGINE == "gpsimd" else nc.vector

        def emit_pred_mms(s, hs):
            p = pprp.tile([OUT_DIM, SW], F32, tag="ppr", name="ppr")
            nc.tensor.matmul(p[:], wout[0][:, :], hs[0][:],
                             start=True, stop=False)
            nc.tensor.matmul(p[:], wout[1][:, :], hs[1][:],
                             start=False, stop=True)
            return p

        def emit_pred_copy(t, s, p):
            """Copy pred psum -> SBUF bf16 (s0 on DVE, s1 on ACT) + DMA out."""
            bs = slice(s * SW, (s + 1) * SW)
            pr = wp.tile([OUT_DIM, SW], BF16, tag=f"pred_{s}", name="pr")
            if s == 0:
                nc.vector.tensor_copy(pr[:], p[:])
            else:
                nc.scalar.copy(pr[:], p[:])
            nc.sync.dma_start(out[t][:, bs], pr[:])
            return pr

        def emit_pred(t, s, hs):
            """pred(t) = W_out @ h'(t); b_out is added host-side."""
            return emit_pred_copy(t, s, emit_pred_mms(s, hs))

        # Software-pipelined main loop (modulo-scheduled): per step the PE
        # queue is [pred+rz+hn](s0), [pred+rz](s1), xn(s0), [hn](s1),
        # xn(s1); the pointwise tail of each stream overlaps the other
        # stream's matmul block. pred(t-1) is issued at the top of step
        # t's block (same readiness point as the step-t gate matmuls).
        def emit_rz_mms(t, s, wk):
            """r/z gate matmuls; ACT sigmoids are emitted separately so the
            ACT FIFO order can be tuned (defer z when needed)."""
            ps = [None] * 4
            for g in range(4):  # r0 r1 z0 z1
                col = slice(g * 128, (g + 1) * 128)
                p = przp.tile([128, SW], F32, tag="prz", name="prz")
                nc.tensor.matmul(p[:], wk[0][:, col], h[s][0][:],
                                 start=True, stop=False)
                nc.tensor.matmul(p[:], wk[1][:, col], h[s][1][:],
                                 start=False, stop=True)
                ps[g] = p
            return ps

        def emit_gate_act(t, s, ps, gates):
            out = []
            for g in gates:
                gg = wp.tile([128, SW], BF16, tag=f"g{g}_{s}", name="gact")
                nc.scalar.activation(gg[:], ps[g][:], AF.Sigmoid,
                                     bias=brz[g][:, t : t + 1])
                out.append(gg)
            return out

        def emit_hn_stt(t, s, rt, first):
            """hn matmuls straight into the nx psum; STT runs IN-PLACE on
            that bank: px = (px + b_hhn) * r. The hn matmuls set the
            has_written bits so the later xn matmul accumulates."""
            srcs = [None, None]
            for c in range(2):
                col = slice(512 + c * 128, 512 + (c + 1) * 128)
                px = pnxp.tile([128, SW], F32, tag="pnx", name="pnx")
                nc.tensor.matmul(px[:], w1[0][:, col], h[s][0][:],
                                 start=True, stop=False)
                nc.tensor.matmul(px[:], w1[1][:, col], h[s][1][:],
                                 start=False, stop=(True if first else False),
                                 skip_group_check=True)
                nc.vector.scalar_tensor_tensor(
                    px[:], px[:], bhhn[c][:, 0:1], rt[c][:], ALU.add, ALU.mult)
                srcs[c] = px
            return srcs

        def emit_xn(srcs, pr):
            """xn = W_xn @ pred(t-1), accumulated onto the in-place STT psum."""
            outs = [None, None]
            for c in range(2):
                xcol = slice(c * 128, (c + 1) * 128)
                nc.tensor.matmul(srcs[c][:], wxn[:, xcol], pr[:],
                                 start=False, stop=True,
                                 skip_group_check=True)
                outs[c] = srcs[c]
            return outs

        def emit_tanh(t, s, srcs):
            n_t = [None, None]
            for c in range(2):
                nt = wp.tile([128, SW], BF16, tag=f"n_{c}_{s}", name="nt")
                nc.scalar.activation(nt[:], srcs[c][:], AF.Tanh,
                                     bias=bxn[c][:, t : t + 1])
                n_t[c] = nt
            return n_t

        def emit_blend(s, n_t, zt, h_new):
            """blend h' = n + z*(h-n)."""
            for c in range(2):
                dt_ = wp.tile([128, SW], BF16, tag=f"d_{c}_{s}", name="dt")
                sub_eng.tensor_tensor(dt_[:], h[s][c][:], n_t[c][:], ALU.subtract)
                e = wp.tile([128, SW], BF16, tag=f"e_{c}_{s}", name="et")
                nc.vector.tensor_tensor(e[:], zt[c][:], dt_[:], ALU.mult)
                hc = wp.tile([128, SW], BF16, tag=f"h{c}_{s}", name="hn2")
                nc.vector.tensor_tensor(hc[:], e[:], n_t[c][:], ALU.add)
                h_new[s][c] = hc

        for t in range(N_TP):
            first = t == 0
            wk = w0 if first else w1
            h_new = [[None, None] for _ in range(NS)]

            # stream 0 matmul block: ACT order r(s0), z(s0) (ACT is idle
            # while PE chews s0's matmuls, so z fits here without delaying)
            pr0 = None if first else emit_pred(t - 1, 0, h[0])
            ps0 = emit_rz_mms(t, 0, wk)
            rt0 = emit_gate_act(t, 0, ps0, (0, 1))
            zt0 = emit_gate_act(t, 0, ps0, (2, 3))
            srcs0 = emit_hn_stt(t, 0, rt0, first)

            # stream 1 matmul block; xn(s0) wedged mid-block on the PE.
            # ACT order: r(s1), pred-copy(s1), tanh(s0) (z(s1) deferred past
            # the tail of s0 to avoid FIFO head-blocking).
            pp1 = None if first else emit_pred_mms(1, h[1])
            ps1 = emit_rz_mms(t, 1, wk)
            rt1 = emit_gate_act(t, 1, ps1, (0, 1))
            pr1 = None if first else emit_pred_copy(t - 1, 1, pp1)
            if not first:
                srcs0 = emit_xn(srcs0, pr0)
            srcs1 = emit_hn_stt(t, 1, rt1, first)
            if not first:
                srcs1 = emit_xn(srcs1, pr1)

            # tails: tanh(s0) ahead of z(s1) in the ACT FIFO
            n0 = emit_tanh(t, 0, srcs0)
            emit_blend(0, n0, zt0, h_new)
            zt1 = emit_gate_act(t, 1, ps1, (2, 3))
            n1 = emit_tanh(t, 1, srcs1)
            emit_blend(1, n1, zt1, h_new)
            h = h_new

        for s in range(NS):
            emit_pred(N_TP - 1, s, h[s])


_CACHE = {}


def _prep_host(z0, tps_to_pred, W_z0, b_z0, W_ih, b_ih, W_hh, b_hh, W_out, b_out):
    import ml_dtypes

    f = np.float32
    bf = ml_dtypes.bfloat16
    z0 = np.asarray(z0, f)
    tps = np.asarray(tps_to_pred, f)
    W_z0, b_z0 = np.asarray(W_z0, f), np.asarray(b_z0, f)
    W_ih, b_ih = np.asarray(W_ih, f), np.asarray(b_ih, f)
    W_hh, b_hh = np.asarray(W_hh, f), np.asarray(b_hh, f)
    W_out, b_out = np.asarray(W_out, f), np.asarray(b_out, f)

    G2 = 2 * N_GRU
    Wihp = W_ih[:, :OUT_DIM]  # [768, 64]
    wt = W_ih[:, OUT_DIM]  # [768]
    Weff_rz = W_hh[:G2] + Wihp[:G2] @ W_out  # [512, 256]
    W1 = np.concatenate([Weff_rz, W_hh[G2:]], axis=0)  # [768, 256]
    w1t = np.ascontiguousarray(W1.T).astype(bf)  # [256, 768]
    wxnt = np.ascontiguousarray(Wihp[G2:].T).astype(bf)  # [64, 256]
    w0t = np.ascontiguousarray(W_hh[:G2].T).astype(bf)  # [256, 512]
    woutt = np.ascontiguousarray(W_out.T).astype(bf)  # [256, 64]
    wz0t = np.ascontiguousarray(W_z0.T).astype(bf)  # [128, 256]

    cb = Wihp @ b_out  # [768]
    bias_all = b_ih[:, None] + wt[:, None] * tps[None, :]  # [768, 200]
    brz = bias_all[:G2] + b_hh[:G2, None]
    brz[:, 1:] += cb[:G2, None]
    bxn = bias_all[G2:].copy()
    bxn[:, 1:] += cb[G2:, None]

    shared = {
        "wz0t": wz0t,
        "w1t": w1t,
        "wxnt": wxnt,
        "w0t": w0t,
        "woutt": woutt,
        "brz": np.ascontiguousarray(brz, f),
        "bxn": np.ascontiguousarray(bxn, f),
        "bhhn": np.ascontiguousarray(b_hh[G2:].reshape(N_GRU, 1)),
        "bz0": np.ascontiguousarray(b_z0.reshape(N_GRU, 1)),
    }
    z0f = z0.reshape(B_FULL, LATENT)
    in_maps = []
    for i in range(N_CORES):
        m = dict(shared)
        m["z0t"] = np.ascontiguousarray(z0f[i * B_LOC : (i + 1) * B_LOC].T).astype(bf)
        in_maps.append(m)
    return in_maps, b_out


def _run(in_maps, **spmd_kwargs):
    if "nc" not in _CACHE:
        _CACHE["nc"] = _build_module()
    return run_bass_kernel_spmd(_CACHE["nc"], in_maps, list(range(N_CORES)), **spmd_kwargs)


def _gather(res, b_out):
    outp = np.empty((B_FULL, N_TP, OUT_DIM), np.float32)
    for i in range(N_CORES):
        o = np.asarray(res.results[i]["out"]).astype(np.float32)  # [200, 64, 1024]
        outp[i * B_LOC : (i + 1) * B_LOC] = o.transpose(2, 0, 1)
    outp += b_out[None, None, :]
    return outp.reshape(64, 128, N_TP, OUT_DIM)


def kernel(**inputs):
    in_maps, b_out = _prep_host(**inputs)
    res = _run(in_maps)
    return _gather(res, b_out)


def kernel_profiled(**inputs):
    """Like kernel(), but requests an NTFF trace; returns (output, results)."""
    in_maps, b_out = _prep_host(**inputs)
    res = _run(in_maps, trace=True)
    return _gather(res, b_out), res
